# revision 35
# baseline (speedup 1.0000x reference)
"""GroupLinear Trainium2 kernel.

out[b, g, o] = sum_i x[b, i] * W[g, o, i] + b[g, o]
  x: (4096, 1024) f32, W: (16, 1024, 1024) f32, b: (16, 1024) f32
  out: (4096, 16, 1024) f32

Sharding: groups across the 8 cores (2 groups/core), x replicated.

Per-core schedule (all operands pre-transposed host-side, cast to bf16;
rel err ~2e-3 vs the 2e-2 gate):
  - x is staged as xt[m][i_part, kt*128 + b] (contraction dim on partitions,
    m-pair tiles for 4KB DMA rows), W as wt[i_part, kt, g*1024+o]. No
    on-device transposes at all.
  - For each batch tile m: 8 kt-steps x 4 psum chunks of 512 accumulate
    out[b, go] with x-tile stationary / W moving (512-wide bf16 matmuls run
    at the 216ns/MM issue roofline); bias fused into the DVE PSUM->SBUF
    evacuation; stores split by partition halves across the two hardware
    DMA queues (sync + scalar).
  - Startup: W chunks stream on sync while dummy matmuls warm the PE HAM
    clock-gate to 8/8; bias is fetched as a single 4KB row and broadcast
    across partitions with K=1 ones-vector matmuls (a [128, GO] broadcast
    DMA would steal the HBM bandwidth that paces the W chunks).
  - A post-finalize pass deletes InstLdweights that reload the stationary
    operand already resident in the PE array (walrus emits one per matmul;
    consecutive matmuls here share lhsT so 3 of 4 reloads are redundant).
"""

import sys
import types

sys.path.insert(0, "/opt/trn_rl_repo")

# Provide antenv.axon_hooks (NTFF profile hook registry) if the installed
# antenv lacks it — must exist before the first jax/axon backend init.
try:
    from antenv import axon_hooks as _axon_hooks  # noqa: F401
except ImportError:
    _m = types.ModuleType("antenv.axon_hooks")
    _m._hook = None

    def _set_hook(hook, _m=_m):
        _m._hook = hook

    def _get_hook(_m=_m):
        return _m._hook

    _m.set_axon_ntff_profile_hook = _set_hook
    _m.get_axon_ntff_profile_hook = _get_hook
    sys.modules["antenv.axon_hooks"] = _m
    try:
        import antenv

        antenv.axon_hooks = _m
    except ImportError:
        pass

from contextlib import ExitStack

import ml_dtypes
import numpy as np

import concourse.mybir as mybir
import concourse.tile as tile
from concourse import bacc
from concourse.bass_utils import run_bass_kernel_spmd

F32 = mybir.dt.float32
BF16 = mybir.dt.bfloat16

BATCH, D_IN, D_OUT, GROUPS, NCORES = 4096, 1024, 1024, 16, 8
GPC = GROUPS // NCORES  # groups per core
P = 128
KT = D_IN // P          # contraction tiles
MT = BATCH // P         # batch tiles
GO = GPC * D_OUT        # output columns per core
CW = 512                # psum chunk width (1 bank fp32)
NCH = GO // CW          # psum chunks per batch tile


def dedup_ldweights(nc):
    """Delete InstLdweights that reload the exact stationary operand already
    resident in the PE array (same AP as the previous PE Ldweights, nothing
    in between that could clobber the array), when they carry no syncs."""
    n_removed = 0
    for blk in nc.m.functions[0].blocks:
        last_key = None
        to_remove = []
        for inst in blk.instructions:
            if getattr(inst, "engine", None) != mybir.EngineType.PE:
                continue
            tn = type(inst).__name__
            if tn == "InstLdweights":
                ap = inst.ins[0]
                key = (ap.memref, ap.offset, str(ap.ap), str(ap.dtype))
                si = inst.sync_info
                has_sync = si is not None and (
                    len(si.on_wait) > 0 or len(si.on_update) > 0
                )
                if key == last_key and not has_sync:
                    to_remove.append(inst)
                    n_removed += 1
                last_key = key
            elif tn == "InstMatmult" and inst.ldweights is False:
                continue  # non-self-loading matmul: array weights unchanged
            elif tn == "InstEventSemaphore":
                continue  # pure sync, does not touch the array
            else:
                last_key = None  # anything else: conservatively assume clobber
        for inst in to_remove:
            blk.instructions.remove(inst)
    return n_removed


def build_nc():
    nc = bacc.Bacc("TRN2", target_bir_lowering=False, debug=False)
    x = nc.dram_tensor("x", [MT // 2, P, 2 * KT * P], BF16, kind="ExternalInput").ap()
    W = nc.dram_tensor("W", [P, KT, GO], BF16, kind="ExternalInput").ap()
    b = nc.dram_tensor("b", [1, GO], BF16, kind="ExternalInput").ap()
    out = nc.dram_tensor("out", [BATCH, GO], F32, kind="ExternalOutput").ap()

    with ExitStack() as ctx:
        tc = ctx.enter_context(tile.TileContext(nc))
        singles = ctx.enter_context(tc.tile_pool(name="singles", bufs=1))
        wt_pool = ctx.enter_context(tc.tile_pool(name="wt", bufs=1))
        xin_pool = ctx.enter_context(tc.tile_pool(name="xin", bufs=3))  # m-pair tiles
        out_pool = ctx.enter_context(tc.tile_pool(name="outp", bufs=3))
        ps_mm = ctx.enter_context(tc.tile_pool(name="ps_mm", bufs=8, space="PSUM"))

        # W resident in SBUF, kt-chunked loads on the sync queue; subtile
        # dependency tracking lets matmuls chase the chunks as they land.
        # Keeping all other startup DMA traffic tiny maximizes the W queue's
        # share of HBM bandwidth (per-queue share is what paces the chunks).
        wt = wt_pool.tile([P, KT, GO], BF16)
        for kt in range(KT):
            nc.sync.dma_start(out=wt[:, kt, :], in_=W[:, kt, :])

        # bias: fetch only the 4KB bf16 row (1-partition transfer on the
        # scalar hardware queue — near-free), then broadcast it to all 128
        # partitions with K=1 ones-vector matmuls while the PE is idle.
        bias_row = singles.tile([1, GO], BF16)
        nc.scalar.dma_start(out=bias_row[:, :], in_=b[:, :])
        ones = singles.tile([1, P], BF16)
        nc.vector.memset(ones[:, :], 1.0)
        bias_sb = singles.tile([P, GO], F32)

        # x loaded as m-pair tiles: 4KB contiguous per-partition rows halve
        # the per-m descriptor load on the scalar hardware DMA queue
        NPAIR = MT // 2

        def load_xpair(pr):
            x_sb = xin_pool.tile([P, 2 * KT * P], BF16, tag="xin")
            nc.scalar.dma_start(out=x_sb[:, :], in_=x[pr, :, :])
            return x_sb

        x_pairs = {0: load_xpair(0)}

        # HAM warmup first: the PE queue is in-order, so these must precede
        # the bias matmuls (which wait on the bias-row DMA) or the PE idles
        # at the queue head. Keeps the PE busy past the 3.4us HAM activity
        # window so the clock-gate is 8/8 before the real matmuls start.
        warm = singles.tile([P, CW], BF16)
        nc.vector.memset(warm[:, :], 0.0)
        warm_ps = ps_mm.tile([P, CW], F32, tag="ps", name="ps_warm")
        for _ in range(8):
            nc.tensor.matmul(
                warm_ps[:, :], warm[:, 0:P], warm[:, :], start=True, stop=True
            )

        bias_pss = [
            ps_mm.tile([P, CW], F32, tag="ps", name=f"ps_bias_{c}")
            for c in range(NCH)
        ]
        for c in range(NCH):
            nc.tensor.matmul(
                bias_pss[c][:, :],
                ones[:, :],
                bias_row[:, c * CW : (c + 1) * CW],
                start=True,
                stop=True,
            )
        for c in range(NCH):
            nc.vector.tensor_copy(
                out=bias_sb[:, c * CW : (c + 1) * CW], in_=bias_pss[c][:, :]
            )

        HP = P // 2

        # Pair 0 runs kt-major across BOTH batch tiles (m0 on 4 psum banks,
        # m1 on the other 4): during the W-chase phase each arriving kt
        # chunk feeds 8 matmuls instead of 4, absorbing chunk-arrival gaps.
        x_sb0 = x_pairs[0]
        pss01 = [
            [
                ps_mm.tile([P, CW], F32, tag="ps", name=f"ps_p0_{h}_{c}")
                for c in range(NCH)
            ]
            for h in range(2)
        ]
        x_pairs[1] = load_xpair(1)
        for kt in range(KT):
            for h in range(2):
                lhsT = x_sb0[:, h * KT * P + kt * P : h * KT * P + (kt + 1) * P]
                for c in range(NCH):
                    nc.tensor.matmul(
                        pss01[h][c][:, :],
                        lhsT,
                        wt[:, kt, c * CW : (c + 1) * CW],
                        start=(kt == 0),
                        stop=(kt == KT - 1),
                    )
        for h in range(2):
            out_sb0 = out_pool.tile([P, GO], F32, tag="outp")
            for c in range(NCH):
                nc.vector.tensor_add(
                    out=out_sb0[:, c * CW : (c + 1) * CW],
                    in0=pss01[h][c][:, :],
                    in1=bias_sb[:, c * CW : (c + 1) * CW],
                )
            nc.sync.dma_start(out=out[h * P : h * P + HP, :], in_=out_sb0[0:HP, :])
            nc.scalar.dma_start(
                out=out[h * P + HP : (h + 1) * P, :], in_=out_sb0[HP:P, :]
            )
        x_pairs.pop(0)

        for m in range(2, MT):
            pr, half = divmod(m, 2)
            if half == 0 and pr + 1 < NPAIR:
                x_pairs[pr + 1] = load_xpair(pr + 1)
            x_sb = x_pairs[pr] if half == 0 else x_pairs.pop(pr)
            base = half * KT * P
            pss = [
                ps_mm.tile([P, CW], F32, tag="ps", name=f"ps_{m}_{c}")
                for c in range(NCH)
            ]
            last = m == MT - 1
            # The last tile computes as two c-pair half-groups so its first
            # column half finishes ~3.5us early; its adds + column-half
            # store then overlap the remaining matmuls, leaving only the
            # second column-half store exposed after the final matmul.
            cgroups = [(0, 1), (2, 3)] if last else [tuple(range(NCH))]
            for cg in cgroups:
                for kt in range(KT):
                    lhsT = x_sb[:, base + kt * P : base + (kt + 1) * P]
                    for c in cg:
                        nc.tensor.matmul(
                            pss[c][:, :],
                            lhsT,
                            wt[:, kt, c * CW : (c + 1) * CW],
                            start=(kt == 0),
                            stop=(kt == KT - 1),
                        )
            out_sb = out_pool.tile([P, GO], F32, tag="outp")
            for cg in cgroups:
                for c in cg:
                    nc.vector.tensor_add(
                        out=out_sb[:, c * CW : (c + 1) * CW],
                        in0=pss[c][:, :],
                        in1=bias_sb[:, c * CW : (c + 1) * CW],
                    )
                # store partition halves concurrently on the two hardware
                # DMA queues (gpsimd's software-descriptor path is ~2x
                # slower per row and would hold the final drain hostage);
                # for the last tile, per c-pair column slices
                lo = cg[0] * CW
                hi = (cg[-1] + 1) * CW
                nc.sync.dma_start(
                    out=out[m * P : m * P + HP, lo:hi], in_=out_sb[0:HP, lo:hi]
                )
                nc.scalar.dma_start(
                    out=out[m * P + HP : (m + 1) * P, lo:hi],
                    in_=out_sb[HP:P, lo:hi],
                )

    nc.finalize()
    dedup_ldweights(nc)
    return nc


_NC_CACHE = {}


def _get_nc():
    if "nc" not in _NC_CACHE:
        _NC_CACHE["nc"] = build_nc()
    return _NC_CACHE["nc"]


def _prep_x(x):
    # x (4096, 1024) f32 -> xt[m, p, kt*128 + c] = x[m*128+c, kt*128+p], then
    # pack m-pairs so each DMA moves 4KB-contiguous per-partition rows
    xt = x.reshape(MT, P, KT, P).transpose(0, 3, 2, 1).reshape(MT, P, KT * P)
    xp = xt.reshape(MT // 2, 2, P, KT * P).transpose(0, 2, 1, 3)
    return np.ascontiguousarray(
        xp.reshape(MT // 2, P, 2 * KT * P).astype(ml_dtypes.bfloat16)
    )


def _prep_w(Wc):
    # Wc (GPC, 1024, 1024) [g, o, i] -> wt[p, kt, g*1024+o], i = kt*128+p, bf16
    wt = Wc.transpose(2, 0, 1).reshape(KT, P, GO)
    return np.ascontiguousarray(wt.transpose(1, 0, 2).astype(ml_dtypes.bfloat16))


def _run(inputs, trace=False):
    x = np.asarray(inputs["x"], dtype=np.float32)
    W = np.asarray(inputs["W"], dtype=np.float32)
    b = np.asarray(inputs["b"], dtype=np.float32)
    nc = _get_nc()
    xt = _prep_x(x)
    in_maps = []
    for c in range(NCORES):
        in_maps.append(
            {
                "x": xt,
                "W": _prep_w(W[c * GPC : (c + 1) * GPC]),
                "b": np.ascontiguousarray(
                    b[c * GPC : (c + 1) * GPC].reshape(1, GO).astype(ml_dtypes.bfloat16)
                ),
            }
        )
    res = run_bass_kernel_spmd(nc, in_maps, core_ids=list(range(NCORES)), trace=trace)
    shards = [r["out"].reshape(BATCH, GPC, D_OUT) for r in res.results]
    return np.concatenate(shards, axis=1), res


def kernel(**inputs):
    out, _ = _run(inputs, trace=False)
    return out
